# revision 1
# baseline (speedup 1.0000x reference)
"""Trainium2 Bass kernel for nn_CS_MAMBA (pool -> mamba -> channel-attention -> FFN).

Data-parallel over batch: 64 batch items sharded 8-per-core across 8 NeuronCores;
all weights replicated. The 8 per-core batch items are processed in groups of
GB: pooling + mamba of group g+1 overlap the FFN matmuls of group g, keeping
the PE busy. fp32 everywhere except the big FFN matmuls and a few
mamba-internal tensors (bf16 operands, fp32 PSUM accumulation).
"""

import numpy as np
import ml_dtypes

# ---------------------------------------------------------------- constants
B_FULL = 64
N_CORES = 8
BL = B_FULL // N_CORES          # 8 batch items per core
GB = 4                          # batch-group size for the pipelined front-end
NG = BL // GB
C = 2048
NCT = C // 128                  # 16 channel tiles
H, W = 24, 12
HW = H * W                      # 288
POOL_W = 48                     # elements summed per patch (4 rows x 12 cols)
L = 12                          # interleaved sequence length
COLSG = L * GB                  # group-local columns, col = l*GB + j
DI = 256                        # d_inner
DIT = DI // 128                 # 2 d_inner tiles
DS = 16                         # d_state
DTR = 16                        # dt_rank
EPS = 1e-5

# packed per-partition small constants: name -> number of [128, n] columns
SMALLS = [
    ("wx", DIT * 48),       # Wx.T as [128, 2, 48]
    ("cw", DIT * 3),        # conv w as [128, 2, 3]
    ("ncb", DIT),           # -conv_b
    ("bdt", DIT),
    ("dssm", DIT),
    ("A3", DIT * DS),       # -exp(A_log) as [128, 2, 16]
    ("ln1g", NCT), ("ln1b", NCT), ("ln2g", NCT), ("ln2b", NCT),
    ("absm", 1), ("absx", 1), ("abnb", 1),
    ("fvs", NCT), ("fvb", NCT), ("fis", NCT), ("fib", NCT),
]
SM_OFF = {}
_off = 0
for _n, _w in SMALLS:
    SM_OFF[_n] = (_off, _off + _w)
    _off += _w
SM_COLS = _off

_CACHE = {}


def _build(nc_mod, tile_mod, mybir, masks, repeat=1, parts="all"):
    """Emit the bass program. Returns the compiled Bass object."""
    F32 = mybir.dt.float32
    BF16 = mybir.dt.bfloat16
    AF = mybir.ActivationFunctionType
    ALU = mybir.AluOpType
    AX = mybir.AxisListType

    nc = nc_mod.Bacc("TRN2", target_bir_lowering=False, debug=False)

    # ---------------- dram tensors (names = in_map keys)
    d_vis = nc.dram_tensor("vis", [BL, C, HW], F32, kind="ExternalInput")
    d_inf = nc.dram_tensor("inf", [BL, C, HW], F32, kind="ExternalInput")
    d_sm = nc.dram_tensor("smalls", [128, SM_COLS], F32, kind="ExternalInput")
    d_winT = nc.dram_tensor("w_inT", [128, NCT, DI], BF16, kind="ExternalInput")
    d_wdtT = nc.dram_tensor("wdtT", [DTR, DI], F32, kind="ExternalInput")
    d_woutT = nc.dram_tensor("w_outT", [128, DIT, C], F32, kind="ExternalInput")
    d_aw1T = nc.dram_tensor("aw1T", [128, NCT, 128], F32, kind="ExternalInput")
    d_aw2T = nc.dram_tensor("aw2T", [128, C], BF16, kind="ExternalInput")
    d_wvT = nc.dram_tensor("wvT", [128, NCT, C], BF16, kind="ExternalInput")
    d_wiT = nc.dram_tensor("wiT", [128, NCT, C], BF16, kind="ExternalInput")

    d_out_vis = nc.dram_tensor("out_vis", [BL, C, HW], F32, kind="ExternalOutput")
    d_out_inf = nc.dram_tensor("out_inf", [BL, C, HW], F32, kind="ExternalOutput")

    with tile_mod.TileContext(nc) as tc:
        with (
            tc.tile_pool(name="consts", bufs=1) as consts,
            tc.tile_pool(name="wpool", bufs=1) as wpool,
            tc.tile_pool(name="axpool", bufs=3) as axpool,
            tc.tile_pool(name="stream", bufs=5) as stream,
            tc.tile_pool(name="streamp", bufs=2) as streamp,
            tc.tile_pool(name="outp", bufs=3) as outp,
            tc.tile_pool(name="vip", bufs=1) as vip,
            tc.tile_pool(name="mam", bufs=1) as mam,
            tc.tile_pool(name="psA", bufs=4, space="PSUM") as psA,
            tc.tile_pool(name="psB", bufs=1, space="PSUM") as psB,
            tc.tile_pool(name="psC", bufs=2, space="PSUM") as psC,
        ):
            # ---------------- constants / weights to SBUF (SWDGE ring)
            ident = consts.tile([128, 128], F32)
            masks.make_identity(nc, ident)
            ones_col = consts.tile([128, 1], F32)
            nc.vector.memset(ones_col, 1.0)
            ones_col_bf = consts.tile([128, 1], BF16)
            nc.vector.memset(ones_col_bf, 1.0)
            ones_row = consts.tile([1, 128], F32)
            nc.vector.memset(ones_row, 1.0)
            epsv = consts.tile([128, 1], F32)
            nc.vector.memset(epsv, EPS)

            sm = consts.tile([128, SM_COLS], F32)
            nc.gpsimd.dma_start(out=sm, in_=d_sm[:, :])

            def smv(name, i3=None):
                a, b = SM_OFF[name]
                v = sm[:, a:b]
                if i3 is not None:
                    v = v.rearrange("p (i k) -> p i k", i=i3)
                return v

            wxT = smv("wx", DIT)
            cw3 = smv("cw", DIT)
            ncb = smv("ncb")
            bdt2 = smv("bdt")
            dssm2 = smv("dssm")
            A3 = smv("A3", DIT)
            ln1g, ln1b = smv("ln1g"), smv("ln1b")
            ln2g, ln2b = smv("ln2g"), smv("ln2b")
            absm, absx, abnb = smv("absm"), smv("absx"), smv("abnb")
            fvs, fvb = smv("fvs"), smv("fvb")
            fis, fib = smv("fis"), smv("fib")

            winT = consts.tile([128, NCT, DI], BF16)
            nc.gpsimd.dma_start(out=winT, in_=d_winT[:, :, :])
            wdtT = consts.tile([DTR, DI], F32)
            nc.gpsimd.dma_start(out=wdtT, in_=d_wdtT[:, :])
            woutT = consts.tile([128, DIT, C], F32)
            nc.gpsimd.dma_start(out=woutT, in_=d_woutT[:, :, :])
            aw1T = consts.tile([128, NCT, 128], F32)
            nc.gpsimd.dma_start(out=aw1T, in_=d_aw1T[:, :, :])
            aw2T = consts.tile([128, C], BF16)
            nc.gpsimd.dma_start(out=aw2T, in_=d_aw2T[:, :])

            fm_d = [d_vis, d_inf]
            out_d = [d_out_vis, d_out_inf]

            import contextlib
            rep_ctx = tc.For_i(0, repeat, 1) if repeat > 1 else contextlib.nullcontext()
            with rep_ctx:
                # per-group attention weights [128, NCT, GB] per stream
                att_g = [
                    [consts.tile([128, NCT, GB], F32, name=f"att{g}_{s}") for s in range(2)]
                    for g in range(NG)
                ]

                # ============================================================
                # Front-end per batch-group g: pool + mamba + attention.
                # Group-local column index = l*GB + j  (j = b - g*GB, l = 2*p + s).
                # ============================================================
                def layer_norm(src_tile, g_tile, b_tile, dst_tile, gi):
                    """LN over channels (partition dim across 16 tiles) per col."""
                    s1p = psC.tile([128, COLSG], F32, tag="ps96", name=f"s1p{gi}")
                    s2p = psC.tile([128, COLSG], F32, tag="ps96", name=f"s2p{gi}")
                    sq = mam.tile([128, NCT, COLSG], BF16, tag="lnsq", name="lnsq")
                    nc.scalar.activation(
                        out=sq[:, :, :], in_=src_tile[:, :, :], func=AF.Square
                    )
                    for ci in range(NCT):
                        nc.tensor.matmul(
                            s1p[0:1, :], ones_col, src_tile[:, ci, :],
                            start=(ci == 0), stop=(ci == NCT - 1),
                        )
                        nc.tensor.matmul(
                            s2p[0:1, :], ones_col_bf, sq[:, ci, :],
                            start=(ci == 0), stop=(ci == NCT - 1),
                        )
                    m_sb = mam.tile([1, COLSG], F32, tag="lnm", name="lnm")
                    nc.vector.tensor_scalar_mul(m_sb, s1p[0:1, :], 1.0 / C)
                    v_sb = mam.tile([1, COLSG], F32, tag="lnv", name="lnv")
                    nc.vector.tensor_scalar_mul(v_sb, s2p[0:1, :], 1.0 / C)
                    msq = mam.tile([1, COLSG], F32, tag="lnmsq", name="lnmsq")
                    nc.vector.tensor_mul(msq, m_sb, m_sb)
                    nc.vector.tensor_sub(v_sb, v_sb, msq)
                    # r = (var+eps)^-1/2 = exp(-0.5*ln(var+eps))
                    r_sb = mam.tile([1, COLSG], F32, tag="lnr", name="lnr")
                    nc.scalar.activation(out=r_sb, in_=v_sb, func=AF.Ln, bias=epsv[0:1, :])
                    nc.scalar.activation(out=r_sb, in_=r_sb, func=AF.Exp, scale=-0.5)
                    mr_sb = mam.tile([1, COLSG], F32, tag="lnmr", name="lnmr")
                    nc.vector.tensor_mul(mr_sb, m_sb, r_sb)
                    rb = psC.tile([128, COLSG], F32, tag="ps96", name=f"lnrb{gi}")
                    nc.tensor.matmul(rb, ones_row, r_sb, start=True, stop=True)
                    mrb = psC.tile([128, COLSG], F32, tag="ps96", name=f"lnmrb{gi}")
                    nc.tensor.matmul(mrb, ones_row, mr_sb, start=True, stop=True)
                    t = mam.tile([128, NCT, COLSG], F32, tag="lnt", name="lnt")
                    rb_bc = rb.unsqueeze(1).broadcast_to([128, NCT, COLSG])
                    mrb_bc = mrb.unsqueeze(1).broadcast_to([128, NCT, COLSG])
                    g_bc = g_tile.unsqueeze(2).broadcast_to([128, NCT, COLSG])
                    b_bc = b_tile.unsqueeze(2).broadcast_to([128, NCT, COLSG])
                    nc.vector.tensor_tensor(
                        out=t[:, :, :], in0=src_tile[:, :, :], in1=rb_bc, op=ALU.mult
                    )
                    nc.vector.tensor_tensor(
                        out=t[:, :, :], in0=t[:, :, :], in1=mrb_bc, op=ALU.subtract
                    )
                    nc.vector.tensor_tensor(
                        out=t[:, :, :], in0=t[:, :, :], in1=g_bc, op=ALU.mult
                    )
                    nc.vector.tensor_tensor(
                        out=dst_tile[:, :, :], in0=t[:, :, :], in1=b_bc, op=ALU.add
                    )

                def pool_group(g):
                    # ---- Phase P: pooled sums into Vi_g (values are 48x pooled avg)
                    Vi = vip.tile([128, NCT, COLSG], F32, tag="vi", bufs=NG, name=f"Vi{g}")
                    for j in range(GB):
                        b = g * GB + j
                        for s in range(2):
                            for cq in range(NCT // 4):
                                ft = streamp.tile(
                                    [128, 4, HW], F32, tag="fmp",
                                    name=f"pfm{g}_{s}_{j}_{cq}",
                                )
                                eng = (
                                    nc.sync if (g == 0 and (cq % 2 == 1)) else nc.scalar
                                )
                                eng.dma_start(
                                    out=ft,
                                    in_=fm_d[s][b, cq * 512 : (cq + 1) * 512, :].rearrange(
                                        "(p four) w -> p four w", four=4
                                    ),
                                )
                                nc.vector.reduce_sum(
                                    out=Vi[:, cq * 4 : (cq + 1) * 4, :].rearrange(
                                        "p c (pp x) -> p c pp x", x=2 * GB
                                    )[:, :, :, GB * s + j],
                                    in_=ft.rearrange(
                                        "p c (pp w) -> p c pp w", w=POOL_W
                                    ),
                                    axis=AX.X,
                                )
                    return Vi

                def mamba_group(g, Vi):
                    # ---- LN1 (scale-invariant w.r.t. the 48x factor)
                    xn = vip.tile([128, NCT, COLSG], BF16, tag="xn", bufs=2, name=f"xn{g}")
                    layer_norm(Vi, ln1g, ln1b, xn, f"a{g}")

                    # ---- x = xn @ W_in.T -> [256(d), cols]; depthwise conv; silu
                    xact = mam.tile([128, DIT, COLSG], F32, tag="xact", name=f"xact{g}")
                    cv = mam.tile([128, DIT, COLSG], F32, tag="cv", name=f"cv{g}")
                    e_t = mam.tile([128, DIT, COLSG], F32, tag="e_t", name=f"e_t{g}")
                    for i in range(DIT):
                        xp = psC.tile([128, COLSG], F32, tag="ps96", name=f"xp{g}_{i}")
                        for ci in range(NCT):
                            nc.tensor.matmul(
                                xp, winT[:, ci, i * 128 : (i + 1) * 128], xn[:, ci, :],
                                start=(ci == 0), stop=(ci == NCT - 1),
                            )
                        nc.vector.tensor_scalar_mul(
                            out=cv[:, i, :], in0=xp, scalar1=cw3[:, i, 1:2]
                        )
                        x_sb = mam.tile(
                            [128, COLSG], F32, tag="xsb", bufs=2, name=f"xsb{g}_{i}"
                        )
                        nc.vector.tensor_copy(out=x_sb, in_=xp)
                        nc.vector.scalar_tensor_tensor(
                            out=cv[:, i, GB:COLSG], in0=x_sb[:, 0 : COLSG - GB],
                            scalar=cw3[:, i, 0:1], in1=cv[:, i, GB:COLSG],
                            op0=ALU.mult, op1=ALU.add,
                        )
                        nc.vector.scalar_tensor_tensor(
                            out=cv[:, i, 0 : COLSG - GB], in0=x_sb[:, GB:COLSG],
                            scalar=cw3[:, i, 2:3], in1=cv[:, i, 0 : COLSG - GB],
                            op0=ALU.mult, op1=ALU.add,
                        )
                        # silu(cv + conv_b) = (cv+cb)/(1+exp(-(cv+cb))); ncb = -conv_b
                        nc.scalar.activation(
                            out=e_t[:, i, :], in_=cv[:, i, :], func=AF.Exp,
                            scale=-1.0, bias=ncb[:, i : i + 1],
                        )
                    nc.vector.tensor_scalar_add(
                        out=e_t[:, :, :], in0=e_t[:, :, :], scalar1=1.0
                    )
                    nc.vector.reciprocal(out=e_t[:, :, :], in_=e_t[:, :, :])
                    for i in range(DIT):
                        nc.vector.scalar_tensor_tensor(
                            out=xact[:, i, :], in0=cv[:, i, :], scalar=ncb[:, i : i + 1],
                            in1=e_t[:, i, :], op0=ALU.subtract, op1=ALU.mult,
                        )

                    # ---- dbc = x @ Wx.T -> [cols(l,j), 48]
                    dbcp = psC.tile([128, 48], F32, tag="ps96", name=f"dbcp{g}")
                    for i in range(DIT):
                        nc.tensor.matmul(
                            dbcp[0:COLSG, :], xact[:, i, :], wxT[:, i, :],
                            start=(i == 0), stop=(i == DIT - 1),
                        )
                    dbc_sb = mam.tile([COLSG, 48], F32, tag="dbc", name=f"dbc{g}")
                    nc.vector.tensor_copy(out=dbc_sb, in_=dbcp[0:COLSG, :])

                    # ---- delta = softplus(delta_in @ Wdt.T + bdt) -> [256, cols]
                    dtp = psC.tile([128, COLSG], F32, tag="ps96", name=f"dtp{g}")
                    nc.tensor.transpose(
                        dtp[0:DTR, :], dbc_sb[:, 0:DTR], ident[0:COLSG, 0:COLSG]
                    )
                    dT_sb = mam.tile([DTR, COLSG], F32, tag="dT", name=f"dT{g}")
                    nc.vector.tensor_copy(out=dT_sb, in_=dtp[0:DTR, :])
                    delta = mam.tile([128, DIT, COLSG], F32, tag="delta", name=f"delta{g}")
                    for i in range(DIT):
                        dp = psC.tile([128, COLSG], F32, tag="ps96", name=f"dp{g}_{i}")
                        nc.tensor.matmul(
                            dp, wdtT[:, i * 128 : (i + 1) * 128], dT_sb,
                            start=True, stop=True,
                        )
                        nc.scalar.activation(
                            out=delta[:, i, :], in_=dp, func=AF.Exp,
                            bias=bdt2[:, i : i + 1],
                        )
                    nc.scalar.activation(
                        out=delta[:, :, :], in_=delta[:, :, :], func=AF.Ln, bias=1.0
                    )

                    # ---- dA = exp(delta x A): [128, i, (l,j,n)]
                    dA = mam.tile([128, DIT, COLSG * DS], BF16, tag="dA", name=f"dA{g}")
                    for i in range(DIT):
                        nc.vector.tensor_tensor(
                            out=dA[:, i, :].rearrange(
                                "p (l jj n) -> p l jj n", jj=GB, n=DS
                            ),
                            in0=delta[:, i, :]
                            .rearrange("p (l jj) -> p l jj", jj=GB)
                            .unsqueeze(3)
                            .broadcast_to([128, L, GB, DS]),
                            in1=A3[:, i, :]
                            .unsqueeze(1)
                            .unsqueeze(1)
                            .broadcast_to([128, L, GB, DS]),
                            op=ALU.mult,
                        )
                    nc.scalar.activation(out=dA[:, :, :], in_=dA[:, :, :], func=AF.Exp)

                    # ---- dBu = (delta*x) x Bp, Bp gathered + row-broadcast
                    du = mam.tile([128, DIT, COLSG], F32, tag="du", name=f"du{g}")
                    nc.vector.tensor_mul(du[:, :, :], delta[:, :, :], xact[:, :, :])
                    bp_flat = mam.tile(
                        [1, COLSG * DS], F32, tag="flat", bufs=2, name=f"bpf{g}"
                    )
                    nc.scalar.dma_start(
                        out=bp_flat.rearrange("o (pb n) -> o pb n", n=DS),
                        in_=dbc_sb[:, DTR : DTR + DS],
                    )
                    dBu = mam.tile([128, DIT, COLSG * DS], BF16, tag="dBu", name=f"dBu{g}")
                    bpb = psB.tile([128, COLSG * DS], F32, tag="bc", name=f"bpb{g}")
                    for k in range((COLSG * DS + 511) // 512):
                        sl = slice(512 * k, min(COLSG * DS, 512 * (k + 1)))
                        nc.tensor.matmul(
                            bpb[:, sl], ones_row, bp_flat[:, sl], start=True, stop=True
                        )
                    for i in range(DIT):
                        nc.vector.tensor_tensor(
                            out=dBu[:, i, :].rearrange(
                                "p (l jj n) -> p l jj n", jj=GB, n=DS
                            ),
                            in0=du[:, i, :]
                            .rearrange("p (l jj) -> p l jj", jj=GB)
                            .unsqueeze(3)
                            .broadcast_to([128, L, GB, DS]),
                            in1=bpb.rearrange("p (l jj n) -> p l jj n", jj=GB, n=DS),
                            op=ALU.mult,
                        )
                    # ---- Cp broadcast to sbuf
                    cp_flat = mam.tile(
                        [1, COLSG * DS], F32, tag="flat", bufs=2, name=f"cpf{g}"
                    )
                    nc.scalar.dma_start(
                        out=cp_flat.rearrange("o (pb n) -> o pb n", n=DS),
                        in_=dbc_sb[:, DTR + DS : DTR + 2 * DS],
                    )
                    cpb_ps = psB.tile([128, COLSG * DS], F32, tag="bc", name=f"cpp{g}")
                    for k in range((COLSG * DS + 511) // 512):
                        sl = slice(512 * k, min(COLSG * DS, 512 * (k + 1)))
                        nc.tensor.matmul(
                            cpb_ps[:, sl], ones_row, cp_flat[:, sl], start=True, stop=True
                        )
                    cpb = mam.tile([128, COLSG * DS], BF16, tag="cpb", name=f"cpb{g}")
                    nc.vector.tensor_copy(out=cpb, in_=cpb_ps)

                    # ---- selective scan (in place on dBu): H_l += dA_l * H_{l-1}
                    for l in range(1, L):
                        blk = slice(l * GB * DS, (l + 1) * GB * DS)
                        prv = slice((l - 1) * GB * DS, l * GB * DS)
                        sc = mam.tile(
                            [128, DIT, GB * DS], F32, tag="sc", bufs=2, name=f"sc{g}_{l}"
                        )
                        nc.vector.tensor_tensor(
                            out=sc[:, :, :], in0=dA[:, :, blk], in1=dBu[:, :, prv],
                            op=ALU.mult,
                        )
                        nc.vector.tensor_tensor(
                            out=dBu[:, :, blk], in0=dBu[:, :, blk], in1=sc[:, :, :],
                            op=ALU.add,
                        )
                    # y = sum_n H * Cp  (dA reused as scratch)
                    nc.vector.tensor_tensor(
                        out=dA[:, :, :], in0=dBu[:, :, :],
                        in1=cpb.unsqueeze(1).broadcast_to([128, DIT, COLSG * DS]),
                        op=ALU.mult,
                    )
                    y2 = mam.tile([128, DIT, COLSG], F32, tag="y2", name=f"y2{g}")
                    nc.vector.reduce_sum(
                        out=y2[:, :, :],
                        in_=dA.rearrange("p i (c n) -> p i c n", n=DS),
                        axis=AX.X,
                    )
                    # y2 += x * D_ssm
                    for i in range(DIT):
                        nc.vector.scalar_tensor_tensor(
                            out=y2[:, i, :], in0=xact[:, i, :], scalar=dssm2[:, i : i + 1],
                            in1=y2[:, i, :], op0=ALU.mult, op1=ALU.add,
                        )

                    # ---- vi2 = y2 @ W_out.T + Vi/48
                    vi2 = vip.tile(
                        [128, NCT, COLSG], F32, tag="v96", bufs=2, name=f"vi2{g}"
                    )
                    vi2p = psB.tile(
                        [128, NCT, COLSG], F32, tag="bc", name=f"vi2p{g}"
                    )
                    for mc in range(NCT):
                        for i in range(DIT):
                            nc.tensor.matmul(
                                vi2p[:, mc, :],
                                woutT[:, i, mc * 128 : (mc + 1) * 128], y2[:, i, :],
                                start=(i == 0), stop=(i == DIT - 1),
                            )
                    nc.vector.scalar_tensor_tensor(
                        out=vi2[:, :, :], in0=Vi[:, :, :], scalar=1.0 / POOL_W,
                        in1=vi2p[:, :, :], op0=ALU.mult, op1=ALU.add,
                    )

                    # ---- LN2
                    nrm = vip.tile(
                        [128, NCT, COLSG], F32, tag="v96", bufs=2, name=f"nrm{g}"
                    )
                    layer_norm(vi2, ln2g, ln2b, nrm, f"b{g}")

                    # ---- channel attention per stream
                    for s in range(2):
                        mv = mam.tile([128, NCT, GB], F32, tag="mv", name=f"mv{g}{s}")
                        mx = mam.tile([128, NCT, GB], F32, tag="mx", name=f"mx{g}{s}")
                        view = nrm.rearrange(
                            "p c (pp two jj) -> p c two jj pp", two=2, jj=GB
                        )[:, :, s, :, :]
                        nc.vector.reduce_sum(out=mv[:, :, :], in_=view, axis=AX.X)
                        nc.vector.reduce_max(out=mx[:, :, :], in_=view, axis=AX.X)
                        h1m = mam.tile([128, GB], BF16, tag="h1m", name=f"h1m{g}{s}")
                        h1x = mam.tile([128, GB], BF16, tag="h1x", name=f"h1x{g}{s}")
                        for src, dst, sc_ap in ((mv, h1m, absm), (mx, h1x, absx)):
                            hp = psC.tile([128, COLSG], F32, tag="ps96", name=f"hp{g}{s}")
                            for ci in range(NCT):
                                nc.tensor.matmul(
                                    hp[:, 0:GB], aw1T[:, ci, :], src[:, ci, :],
                                    start=(ci == 0), stop=(ci == NCT - 1),
                                )
                            nc.scalar.activation(
                                out=dst, in_=hp[:, 0:GB], func=AF.Relu,
                                scale=sc_ap, bias=abnb,
                            )
                        att = att_g[g][s]
                        apb = psB.tile(
                            [128, NCT, COLSG], F32, tag="bc", name=f"apb{g}{s}"
                        )
                        for mc in range(NCT):
                            nc.tensor.matmul(
                                apb[:, mc, 0:GB], aw2T[:, mc * 128 : (mc + 1) * 128], h1m,
                                start=True, stop=False,
                            )
                            nc.tensor.matmul(
                                apb[:, mc, 0:GB], aw2T[:, mc * 128 : (mc + 1) * 128], h1x,
                                start=False, stop=True,
                            )
                        nc.scalar.activation(
                            out=att[:, :, :], in_=apb[:, :, 0:GB], func=AF.Exp,
                            scale=-1.0,
                        )
                        nc.vector.tensor_scalar_add(
                            out=att[:, :, :], in0=att[:, :, :], scalar1=1.0
                        )
                        nc.vector.reciprocal(out=att[:, :, :], in_=att[:, :, :])

                # ============================================================
                # Phase F: out = relu((W @ (a*fm))*s + b) per stream, per batch.
                # Emission is software-pipelined with the front-end groups.
                # ============================================================
                scl_s = [fvs, fis]
                scl_b = [fvb, fib]
                w_dram = [d_wvT, d_wiT]
                wts = {}

                def load_w(s):
                    wt = [
                        wpool.tile([128, C], BF16, tag="w", bufs=NCT, name=f"w{s}_{kc}")
                        for kc in range(NCT)
                    ]
                    for kc in range(NCT):
                        nc.gpsimd.dma_start(out=wt[kc], in_=w_dram[s][:, kc, :])
                    wts[s] = wt

                def ffn_batch(s, b, skip_io=False):
                    wt = wts[s]
                    att = att_g[b // GB][s]
                    j = b % GB
                    ax = [
                        axpool.tile(
                            [128, HW], BF16, tag="ax", bufs=2 * NCT,
                            name=f"ax{s}_{b}_{kc}",
                        )
                        for kc in range(NCT)
                    ]
                    if skip_io:
                        for kc in range(NCT):
                            nc.gpsimd.memset(ax[kc], 0.01)
                    else:
                        for cq in range(NCT // 4):
                            ft = stream.tile(
                                [128, 4, HW], F32, tag="fm", name=f"ffm{s}_{b}_{cq}"
                            )
                            nc.sync.dma_start(
                                out=ft,
                                in_=fm_d[s][b, cq * 512 : (cq + 1) * 512, :].rearrange(
                                    "(p four) w -> p four w", four=4
                                ),
                            )
                            for ci in range(4):
                                kc = cq * 4 + ci
                                if ci % 2 == 0:
                                    nc.vector.tensor_scalar_mul(
                                        out=ax[kc], in0=ft[:, ci, :],
                                        scalar1=att[:, kc, j : j + 1],
                                    )
                                else:
                                    nc.scalar.activation(
                                        out=ax[kc], in_=ft[:, ci, :], func=AF.Copy,
                                        scale=att[:, kc, j : j + 1],
                                    )
                    for mq in range(NCT // 4):
                        ot = outp.tile(
                            [128, 4, HW], F32, tag="ot", name=f"ot{s}_{b}_{mq}"
                        )
                        for mi in range(4):
                            mc = mq * 4 + mi
                            pp = psA.tile(
                                [128, HW], F32, tag="pp", name=f"pp{s}_{b}_{mc}"
                            )
                            for kc in range(NCT):
                                nc.tensor.matmul(
                                    pp, wt[kc][:, mc * 128 : (mc + 1) * 128],
                                    ax[kc],
                                    start=(kc == 0), stop=(kc == NCT - 1),
                                )
                            nc.scalar.activation(
                                out=ot[:, mi, :], in_=pp, func=AF.Relu,
                                scale=scl_s[s][:, mc : mc + 1],
                                bias=scl_b[s][:, mc : mc + 1],
                            )
                        if not skip_io:
                            nc.sync.dma_start(
                                out=out_d[s][b, mq * 512 : (mq + 1) * 512, :].rearrange(
                                    "(p four) w -> p four w", four=4
                                ),
                                in_=ot,
                            )

                # ---- emission: software-pipelined fronts and FFN stages
                if parts == "front":
                    for g in range(NG):
                        Vi_g = pool_group(g)
                        mamba_group(g, Vi_g)
                elif parts in ("ffn", "mm"):
                    for gg in range(NG):
                        for ss in range(2):
                            nc.vector.memset(att_g[gg][ss], 1.0)
                    load_w(0)
                    for b in range(BL):
                        ffn_batch(0, b, skip_io=(parts == "mm"))
                    load_w(1)
                    for b in range(BL):
                        ffn_batch(1, b, skip_io=(parts == "mm"))
                else:
                    Vi0 = pool_group(0)
                    mamba_group(0, Vi0)
                    load_w(0)
                    for g in range(1, NG):
                        for j in range(GB):
                            ffn_batch(0, (g - 1) * GB + j)
                        Vi_g = pool_group(g)
                        mamba_group(g, Vi_g)
                    for j in range(GB):
                        ffn_batch(0, (NG - 1) * GB + j)
                    load_w(1)
                    for b in range(BL):
                        ffn_batch(1, b)

    nc.compile()
    return nc


# channel permutation: K-tile kc, partition p holds channel (kc//4)*512 + 4*p + (kc%4)
# so each DMA descriptor covers 4 consecutive channels (4.6KB contiguous).
_PERM = np.array(
    [[(kc // 4) * 512 + 4 * p + (kc % 4) for p in range(128)] for kc in range(NCT)]
).reshape(-1)  # [2048] in (kc, p) order


def _host_prep(inputs):
    """Host-side weight layout prep. Returns dict of per-core-replicated arrays."""
    f32 = np.float32
    g = lambda k: np.asarray(inputs[k], dtype=f32)
    s_bn = f32(1.0 / np.sqrt(1.0 + EPS))

    def ctile(v):  # [C] -> [128, 16], channel-permuted
        return np.ascontiguousarray(v[_PERM].reshape(NCT, 128).T)

    def dtile(v):  # [DI] -> [128, 2]
        return np.ascontiguousarray(v.reshape(DIT, 128).T)

    A = -np.exp(g("A_log"))  # [256, 16]
    sm_parts = {
        "wx": g("Wx").T.reshape(DIT, 128, 48).transpose(1, 0, 2).reshape(128, -1),
        "cw": g("conv_w")[:, 0, :].reshape(DIT, 128, 3).transpose(1, 0, 2).reshape(128, -1),
        "ncb": dtile(-g("conv_b")),
        "bdt": dtile(g("bdt")),
        "dssm": dtile(g("D_ssm")),
        "A3": A.reshape(DIT, 128, DS).transpose(1, 0, 2).reshape(128, -1),
        "ln1g": ctile(g("ln1_g")), "ln1b": ctile(g("ln1_b")),
        "ln2g": ctile(g("ln2_g")), "ln2b": ctile(g("ln2_b")),
        "absm": (g("att_bn_g") * s_bn / 6.0)[:, None],
        "absx": (g("att_bn_g") * s_bn)[:, None],
        "abnb": g("att_bn_b")[:, None],
        "fvs": ctile(g("ffn_vis_bn_g") * s_bn),
        "fvb": ctile(g("ffn_vis_b") * (g("ffn_vis_bn_g") * s_bn) + g("ffn_vis_bn_b")),
        "fis": ctile(g("ffn_inf_bn_g") * s_bn),
        "fib": ctile(g("ffn_inf_b") * (g("ffn_inf_bn_g") * s_bn) + g("ffn_inf_bn_b")),
    }
    smalls = np.zeros((128, SM_COLS), f32)
    for name, _w in SMALLS:
        a, b = SM_OFF[name]
        smalls[:, a:b] = sm_parts[name]

    prep = {
        "smalls": smalls,
        "w_inT": np.ascontiguousarray(
            g("W_in").T[_PERM].reshape(NCT, 128, DI).transpose(1, 0, 2)
        ).astype(ml_dtypes.bfloat16),
        "wdtT": np.ascontiguousarray(g("Wdt").T),
        "w_outT": np.ascontiguousarray(
            g("W_out").T[:, _PERM].reshape(DIT, 128, C).transpose(1, 0, 2)
        ),
        "aw1T": np.ascontiguousarray(
            g("att_w1").T[_PERM].reshape(NCT, 128, 128).transpose(1, 0, 2)
        ),
        "aw2T": np.ascontiguousarray(g("att_w2").T[:, _PERM]).astype(
            ml_dtypes.bfloat16
        ),
        "wvT": np.ascontiguousarray(
            g("ffn_vis_w").T[_PERM][:, _PERM].reshape(NCT, 128, C).transpose(1, 0, 2)
        ).astype(ml_dtypes.bfloat16),
        "wiT": np.ascontiguousarray(
            g("ffn_inf_w").T[_PERM][:, _PERM].reshape(NCT, 128, C).transpose(1, 0, 2)
        ).astype(ml_dtypes.bfloat16),
    }
    return prep


def _get_runner():
    """Build the bass program once and wrap it in a reusable jitted callable."""
    if "runner" in _CACHE:
        return _CACHE["runner"]

    import jax
    import numpy as _np
    from jax.sharding import Mesh, PartitionSpec
    from jax.experimental.shard_map import shard_map
    import concourse.bacc as bacc
    import concourse.tile as tile
    from concourse import mybir, masks
    from concourse import bass2jax

    nc = _build(bacc, tile, mybir, masks)
    bass2jax.install_neuronx_cc_hook()

    pname = nc.partition_id_tensor.name if nc.partition_id_tensor else None
    in_names, out_names, out_avals, zero_shapes = [], [], [], []
    for alloc in nc.m.functions[0].allocations:
        if not isinstance(alloc, mybir.MemoryLocationSet):
            continue
        name = alloc.memorylocations[0].name
        if alloc.kind == "ExternalInput":
            if name != pname:
                in_names.append(name)
        elif alloc.kind == "ExternalOutput":
            out_names.append(name)
            shape = tuple(alloc.tensor_shape)
            dtype = mybir.dt.np(alloc.dtype)
            out_avals.append(jax.core.ShapedArray(shape, dtype))
            zero_shapes.append((shape, dtype))
    n_params = len(in_names)
    all_names = list(in_names) + list(out_names)
    if pname is not None:
        all_names.append(pname)

    def _body(*args):
        operands = list(args)
        if pname is not None:
            operands.append(bass2jax.partition_id_tensor())
        outs = bass2jax._bass_exec_p.bind(
            *operands,
            out_avals=tuple(out_avals),
            in_names=tuple(all_names),
            out_names=tuple(out_names),
            lowering_input_output_aliases=(),
            sim_require_finite=False,
            sim_require_nnan=False,
            nc=nc,
        )
        return tuple(outs)

    devices = jax.devices()[:N_CORES]
    mesh = Mesh(_np.asarray(devices), ("core",))
    specs = (PartitionSpec("core"),) * (n_params + len(out_names))
    fn = jax.jit(
        shard_map(
            _body,
            mesh=mesh,
            in_specs=specs,
            out_specs=(PartitionSpec("core"),) * len(out_names),
            check_rep=False,
        ),
        keep_unused=True,
    )
    runner = {
        "fn": fn,
        "in_names": in_names,
        "out_names": out_names,
        "zero_shapes": zero_shapes,
        "nc": nc,
    }
    _CACHE["runner"] = runner
    return runner


def kernel(**inputs):
    runner = _get_runner()
    prep = _host_prep(inputs)
    vis = np.asarray(inputs["vis_feat_map"], dtype=np.float32).reshape(B_FULL, C, HW)
    inf = np.asarray(inputs["inf_feat_map"], dtype=np.float32).reshape(B_FULL, C, HW)

    # global inputs: concat of per-core shards along axis 0
    per_in = {"vis": vis, "inf": inf}  # already [64, ...] = 8 cores x [8, ...]
    gin = []
    for name in runner["in_names"]:
        if name in per_in:
            gin.append(per_in[name])
        else:
            arr = prep[name]
            gin.append(np.broadcast_to(arr, (N_CORES,) + arr.shape).reshape(
                (N_CORES * arr.shape[0],) + arr.shape[1:]
            ))
    zeros = [
        np.zeros((N_CORES * s[0],) + tuple(s[1:]), dt)
        for (s, dt) in runner["zero_shapes"]
    ]
    outs = runner["fn"](*gin, *zeros)
    res = {
        name: np.asarray(outs[i]) for i, name in enumerate(runner["out_names"])
    }
    out_vis = res["out_vis"].reshape(B_FULL, C, H, W)
    out_inf = res["out_inf"].reshape(B_FULL, C, H, W)
    return (out_vis, out_inf)



# revision 39
# speedup vs baseline: 1.4696x; 1.4696x over previous
"""Trainium2 Bass kernel for nn_CS_MAMBA (pool -> mamba -> channel-attention -> FFN).

Data-parallel over batch: 64 batch items sharded 8-per-core across 8 NeuronCores;
all weights replicated. v2 design:
- variable mamba groups [2,3,3] so the serial prologue (pool+mamba of the first
  group) is short;
- all pool DMAs are issued up-front on the sync queue in consumption order with
  FFN feature-map loads interleaved at fixed positions;
- fm tiles are cast to bf16 `ax` staging immediately on landing (so staging
  slots free fast and pool loads never queue behind attention-dependent work);
  the attention gate is applied in-place just before the matmuls;
- FFN matmuls process batch PAIRS (moving operand 576 cols bf16) to halve the
  per-matmul overhead;
- mamba emission is split into 4 chunks hooked between FFN matmul groups so its
  serial vector chains execute under PE work instead of stalling the FIFO
  tensor queue;
- outputs are written as bf16 (host upcasts) to halve store traffic.
"""

import numpy as np
import ml_dtypes

# ---------------------------------------------------------------- constants
B_FULL = 64
N_CORES = 8
BL = B_FULL // N_CORES          # 8 batch items per core
C = 2048
NCT = C // 128                  # 16 channel tiles
H, W = 24, 12
HW = H * W                      # 288
POOL_W = 48                     # elements summed per patch (4 rows x 12 cols)
L = 12                          # interleaved sequence length
DI = 256                        # d_inner
DIT = DI // 128                 # 2 d_inner tiles
DS = 16                         # d_state
DTR = 16                        # dt_rank
EPS = 1e-5

GROUPS = [2, 3, 3]              # mamba batch-group sizes
GSTART = [0, 2, 5]
B2G = [0, 0, 1, 1, 1, 2, 2, 2]
NG = len(GROUPS)
COLS = [L * gb for gb in GROUPS]
CMAX = max(COLS)                # 36
PAIRS = [(0, 1), (2, 3), (4, 5), (6, 7)]

# packed per-partition small constants: name -> number of [128, n] columns
SMALLS = [
    ("wx", DIT * 48),       # Wx.T as [128, 2, 48]
    ("cw", DIT * 3),        # conv w as [128, 2, 3]
    ("ncb", DIT),           # -conv_b
    ("cb", DIT),            # +conv_b
    ("bdt", DIT),
    ("dssm", DIT),
    ("A3", DIT * DS),       # -exp(A_log) as [128, 2, 16]
    ("ln1g", NCT), ("ln1b", NCT), ("ln2g", NCT), ("ln2b", NCT),
    ("absm", 1), ("absx", 1), ("abnb", 1),
    ("fvs", NCT), ("fvb", NCT), ("fis", NCT), ("fib", NCT),
]
SM_OFF = {}
_off = 0
for _n, _w in SMALLS:
    SM_OFF[_n] = (_off, _off + _w)
    _off += _w
SM_COLS = _off

_CACHE = {}


def _build(nc_mod, tile_mod, mybir, masks, repeat=1, parts="all"):
    """Emit the bass program. Returns the compiled Bass object."""
    F32 = mybir.dt.float32
    BF16 = mybir.dt.bfloat16
    AF = mybir.ActivationFunctionType
    ALU = mybir.AluOpType
    AX = mybir.AxisListType

    nc = nc_mod.Bacc("TRN2", target_bir_lowering=False, debug=False)

    # ---------------- dram tensors (names = in_map keys)
    d_vis = nc.dram_tensor("vis", [BL, C, HW], F32, kind="ExternalInput")
    d_inf = nc.dram_tensor("inf", [BL, C, HW], F32, kind="ExternalInput")
    d_sm = nc.dram_tensor("smalls", [128, SM_COLS], F32, kind="ExternalInput")
    d_winT = nc.dram_tensor("w_inT", [128, NCT, DI], BF16, kind="ExternalInput")
    d_wdtT = nc.dram_tensor("wdtT", [DTR, DI], F32, kind="ExternalInput")
    d_woutT = nc.dram_tensor("w_outT", [128, DIT, C], BF16, kind="ExternalInput")
    d_aw1T = nc.dram_tensor("aw1T", [128, NCT, 128], BF16, kind="ExternalInput")
    d_aw2T = nc.dram_tensor("aw2T", [128, C], BF16, kind="ExternalInput")
    d_wvT = nc.dram_tensor("wvT", [128, NCT, C], BF16, kind="ExternalInput")
    d_wiT = nc.dram_tensor("wiT", [128, NCT, C], BF16, kind="ExternalInput")

    d_out_vis = nc.dram_tensor("out_vis", [BL, C, HW], BF16, kind="ExternalOutput")
    d_out_inf = nc.dram_tensor("out_inf", [BL, C, HW], BF16, kind="ExternalOutput")

    fm_d = [d_vis, d_inf]
    out_d = [d_out_vis, d_out_inf]

    with tile_mod.TileContext(nc) as tc:
        with (
            tc.tile_pool(name="consts", bufs=1) as consts,
            tc.tile_pool(name="wpool", bufs=1) as wpool,
            tc.tile_pool(name="fmp", bufs=6) as fmp,
            tc.tile_pool(name="axp", bufs=9) as axp,
            tc.tile_pool(name="otp", bufs=4) as otp,
            tc.tile_pool(name="vip", bufs=1) as vip,
            tc.tile_pool(name="mam", bufs=1) as mam,
            tc.tile_pool(name="psA", bufs=4, space="PSUM") as psA,
            tc.tile_pool(name="psB", bufs=1, space="PSUM") as psB,
            tc.tile_pool(name="psC", bufs=2, space="PSUM") as psC,
        ):
            # ---------------- constants / weights to SBUF
            ident = consts.tile([128, 128], F32)
            masks.make_identity(nc, ident)
            ones_col = consts.tile([128, 1], F32)
            nc.vector.memset(ones_col, 1.0)
            ones_col_bf = consts.tile([128, 1], BF16)
            nc.vector.memset(ones_col_bf, 1.0)
            ones_row = consts.tile([1, 128], F32)
            nc.vector.memset(ones_row, 1.0)
            epsv = consts.tile([128, 1], F32)
            nc.vector.memset(epsv, EPS)

            sm = consts.tile([128, SM_COLS], F32)
            nc.gpsimd.dma_start(out=sm, in_=d_sm[:, :])

            def smv(name, i3=None):
                a, b = SM_OFF[name]
                v = sm[:, a:b]
                if i3 is not None:
                    v = v.rearrange("p (i k) -> p i k", i=i3)
                return v

            wxT = smv("wx", DIT)
            cw3 = smv("cw", DIT)
            ncb = smv("ncb")
            cb2 = smv("cb")
            bdt2 = smv("bdt")
            dssm2 = smv("dssm")
            A3 = smv("A3", DIT)
            ln1g, ln1b = smv("ln1g"), smv("ln1b")
            ln2g, ln2b = smv("ln2g"), smv("ln2b")
            absm, absx, abnb = smv("absm"), smv("absx"), smv("abnb")
            fvs, fvb = smv("fvs"), smv("fvb")
            fis, fib = smv("fis"), smv("fib")

            winT = consts.tile([128, NCT, DI], BF16)
            nc.gpsimd.dma_start(out=winT, in_=d_winT[:, :, :])
            wdtT = consts.tile([DTR, DI], F32)
            nc.gpsimd.dma_start(out=wdtT, in_=d_wdtT[:, :])
            woutT = consts.tile([128, DIT, C], BF16)
            nc.gpsimd.dma_start(out=woutT, in_=d_woutT[:, :, :])
            aw1T = consts.tile([128, NCT, 128], BF16)
            nc.gpsimd.dma_start(out=aw1T, in_=d_aw1T[:, :, :])
            aw2T = consts.tile([128, C], BF16)
            nc.gpsimd.dma_start(out=aw2T, in_=d_aw2T[:, :])

            scl_s = [fvs, fis]
            scl_b = [fvb, fib]
            w_dram = [d_wvT, d_wiT]

            import contextlib
            rep_ctx = tc.For_i(0, repeat, 1) if repeat > 1 else contextlib.nullcontext()
            with rep_ctx:
                # per-group attention gates [128, NCT, gb] per stream
                att_g = [
                    [
                        consts.tile([128, NCT, GROUPS[g]], F32, name=f"att{g}_{s}",
                                    tag="attg", bufs=2 * NG)
                        for s in range(2)
                    ]
                    for g in range(NG)
                ]
                # FFN weights, quartered by output block mq: 68 rotating
                # [128,512] buffers. Stream-0 quarters free at each mq
                # boundary of its last pair, so stream-1 quarters (0.36us
                # transfers each) prefetch with no transition stall.
                wts = {}

                def load_w(s, mqs=(0, 1, 2, 3)):
                    wt = wts.setdefault(s, [None] * 4)
                    for mq in mqs:
                        row = []
                        for kc in range(NCT):
                            t = wpool.tile([128, 512], BF16, tag="wq", bufs=68,
                                           name=f"w{s}_{mq}_{kc}")
                            row.append(t)
                            nc.sync.dma_start(
                                out=t,
                                in_=w_dram[s][:, kc, mq * 512:(mq + 1) * 512],
                            )
                        wt[mq] = row

                # ========================================================
                # layer norm over channels (partition dim across 16 tiles)
                # ========================================================
                def layer_norm(src_tile, g_tile, b_tile, dst_tile, cols, gi,
                               per_ci=False):
                    s1p = psC.tile([128, 48], F32, tag="ps48", name=f"s1p{gi}")
                    s2p = psC.tile([128, 48], F32, tag="ps48", name=f"s2p{gi}")
                    sq = mam.tile([128, NCT, CMAX], BF16, tag="lnsq", name="lnsq")
                    if not per_ci:
                        nc.scalar.activation(
                            out=sq[:, :, :cols], in_=src_tile[:, :, :cols],
                            func=AF.Square,
                        )
                    for ci in range(NCT):
                        if per_ci:
                            nc.scalar.activation(
                                out=sq[:, ci, :cols], in_=src_tile[:, ci, :cols],
                                func=AF.Square,
                            )
                        nc.tensor.matmul(
                            s1p[0:1, :cols], ones_col, src_tile[:, ci, :cols],
                            start=(ci == 0), stop=(ci == NCT - 1),
                        )
                        nc.tensor.matmul(
                            s2p[0:1, :cols], ones_col_bf, sq[:, ci, :cols],
                            start=(ci == 0), stop=(ci == NCT - 1),
                        )
                    m_sb = mam.tile([1, CMAX], F32, tag="lnm", name="lnm")
                    nc.vector.tensor_scalar_mul(m_sb[:, :cols], s1p[0:1, :cols], 1.0 / C)
                    v_sb = mam.tile([1, CMAX], F32, tag="lnv", name="lnv")
                    nc.vector.tensor_scalar_mul(v_sb[:, :cols], s2p[0:1, :cols], 1.0 / C)
                    msq = mam.tile([1, CMAX], F32, tag="lnmsq", name="lnmsq")
                    nc.vector.tensor_mul(msq[:, :cols], m_sb[:, :cols], m_sb[:, :cols])
                    nc.vector.tensor_sub(v_sb[:, :cols], v_sb[:, :cols], msq[:, :cols])
                    # r = (var+eps)^-1/2 = exp(-0.5*ln(var+eps))
                    r_sb = mam.tile([1, CMAX], F32, tag="lnr", name="lnr")
                    nc.scalar.activation(
                        out=r_sb[:, :cols], in_=v_sb[:, :cols], func=AF.Ln,
                        bias=epsv[0:1, :],
                    )
                    nc.scalar.activation(
                        out=r_sb[:, :cols], in_=r_sb[:, :cols], func=AF.Exp, scale=-0.5
                    )
                    mr_sb = mam.tile([1, CMAX], F32, tag="lnmr", name="lnmr")
                    nc.vector.tensor_mul(mr_sb[:, :cols], m_sb[:, :cols], r_sb[:, :cols])
                    rb = psC.tile([128, 48], F32, tag="ps48", name=f"lnrb{gi}")
                    nc.tensor.matmul(
                        rb[:, :cols], ones_row, r_sb[:, :cols], start=True, stop=True
                    )
                    mrb = psC.tile([128, 48], F32, tag="ps48", name=f"lnmrb{gi}")
                    nc.tensor.matmul(
                        mrb[:, :cols], ones_row, mr_sb[:, :cols], start=True, stop=True
                    )
                    t = mam.tile([128, NCT, CMAX], F32, tag="lnt", name="lnt")
                    rb_bc = rb[:, :cols].unsqueeze(1).broadcast_to([128, NCT, cols])
                    mrb_bc = mrb[:, :cols].unsqueeze(1).broadcast_to([128, NCT, cols])
                    g_bc = g_tile.unsqueeze(2).broadcast_to([128, NCT, cols])
                    b_bc = b_tile.unsqueeze(2).broadcast_to([128, NCT, cols])
                    nc.vector.tensor_tensor(
                        out=t[:, :, :cols], in0=src_tile[:, :, :cols], in1=rb_bc,
                        op=ALU.mult,
                    )
                    nc.vector.tensor_tensor(
                        out=t[:, :, :cols], in0=t[:, :, :cols], in1=mrb_bc,
                        op=ALU.subtract,
                    )
                    nc.vector.tensor_tensor(
                        out=t[:, :, :cols], in0=t[:, :, :cols], in1=g_bc, op=ALU.mult
                    )
                    nc.vector.tensor_tensor(
                        out=dst_tile[:, :, :cols], in0=t[:, :, :cols], in1=b_bc,
                        op=ALU.add,
                    )

                # ========================================================
                # pool loads: one [128,4,288] tile per (b, s, cq); reduce
                # into Vi[g] columns (values are 48x the pooled avg).
                # ========================================================
                Vi_g = [None] * NG

                def pool_loads(g, defer_reds=False):
                    """cq-major loads so LN1 channel sums can start before the
                    whole group has landed. Returns deferred reduce thunks if
                    defer_reds (the loads still stream; reduces are emitted
                    later to keep them off the DVE queue head)."""
                    gb = GROUPS[g]
                    cols = COLS[g]
                    Vi = vip.tile([128, NCT, CMAX], F32, tag="vi", bufs=NG,
                                  name=f"Vi{g}")
                    Vi_g[g] = Vi
                    reds = []
                    for cq in range(NCT // 4):
                        for j in range(gb):
                            b = GSTART[g] + j
                            for s in range(2):
                                ft = fmp.tile(
                                    [128, 4, HW], F32, tag="fm",
                                    name=f"pfm{g}_{s}_{j}_{cq}",
                                )
                                nc.sync.dma_start(
                                    out=ft,
                                    in_=fm_d[s][b, cq * 512:(cq + 1) * 512, :]
                                    .rearrange("(p four) w -> p four w", four=4),
                                )

                                def red(ft=ft, cq=cq, j=j, s=s):
                                    nc.vector.reduce_sum(
                                        out=Vi[:, cq * 4:(cq + 1) * 4, :cols]
                                        .rearrange(
                                            "p c (pp x) -> p c pp x", x=2 * gb
                                        )[:, :, :, gb * s + j],
                                        in_=ft.rearrange(
                                            "p c (pp w) -> p c pp w", w=POOL_W
                                        ),
                                        axis=AX.X,
                                    )

                                if defer_reds:
                                    reds.append(red)
                                else:
                                    red()
                    return reds

                # ========================================================
                # mamba chunks (emitted via hooks between FFN mq groups)
                # ========================================================
                mstate = [dict() for _ in range(NG)]

                def mamba_A1(g, per_ci=False):
                    """LN1 + x = xn @ W_in.T + depthwise conv + silu."""
                    gb, cols = GROUPS[g], COLS[g]
                    st = mstate[g]
                    xn = vip.tile([128, NCT, CMAX], BF16, tag="xn", bufs=2,
                                  name=f"xn{g}")
                    layer_norm(Vi_g[g], ln1g, ln1b, xn, cols, f"a{g}",
                               per_ci=per_ci)

                    xact = mam.tile([128, DIT, CMAX], F32, tag="xact", name=f"xact{g}")
                    cv = mam.tile([128, DIT, CMAX], F32, tag="cv", name=f"cv{g}")
                    e_t = mam.tile([128, DIT, CMAX], F32, tag="e_t", name=f"e_t{g}")
                    for i in range(DIT):
                        xp = psC.tile([128, 48], F32, tag="ps48", name=f"xp{g}_{i}")
                        for ci in range(NCT):
                            nc.tensor.matmul(
                                xp[:, :cols], winT[:, ci, i * 128:(i + 1) * 128],
                                xn[:, ci, :cols],
                                start=(ci == 0), stop=(ci == NCT - 1),
                            )
                        nc.vector.tensor_scalar_mul(
                            out=cv[:, i, :cols], in0=xp[:, :cols], scalar1=cw3[:, i, 1:2]
                        )
                        x_sb = mam.tile(
                            [128, CMAX], F32, tag="xsb", bufs=2, name=f"xsb{g}_{i}"
                        )
                        nc.vector.tensor_copy(out=x_sb[:, :cols], in_=xp[:, :cols])
                        nc.vector.scalar_tensor_tensor(
                            out=cv[:, i, gb:cols], in0=x_sb[:, 0:cols - gb],
                            scalar=cw3[:, i, 0:1], in1=cv[:, i, gb:cols],
                            op0=ALU.mult, op1=ALU.add,
                        )
                        nc.vector.scalar_tensor_tensor(
                            out=cv[:, i, 0:cols - gb], in0=x_sb[:, gb:cols],
                            scalar=cw3[:, i, 2:3], in1=cv[:, i, 0:cols - gb],
                            op0=ALU.mult, op1=ALU.add,
                        )
                        # fold +conv_b into cv so silu needs only tensor ops
                        nc.vector.tensor_scalar(
                            out=cv[:, i, :cols], in0=cv[:, i, :cols],
                            scalar1=cb2[:, i:i + 1], scalar2=None, op0=ALU.add,
                        )
                        # silu(cv) = cv/(1+exp(-cv))
                        nc.scalar.activation(
                            out=e_t[:, i, :cols], in_=cv[:, i, :cols], func=AF.Exp,
                            scale=-1.0,
                        )
                    ones_bc = ones_col.unsqueeze(2).broadcast_to([128, DIT, cols])
                    nc.gpsimd.tensor_tensor(
                        out=e_t[:, :, :cols], in0=e_t[:, :, :cols], in1=ones_bc,
                        op=ALU.add,
                    )
                    nc.vector.reciprocal(out=e_t[:, :, :cols], in_=e_t[:, :, :cols])
                    nc.gpsimd.tensor_tensor(
                        out=xact[:, :, :cols], in0=cv[:, :, :cols],
                        in1=e_t[:, :, :cols], op=ALU.mult,
                    )
                    st["xact"] = xact

                def mamba_A2(g):
                    """dbc = x @ Wx.T; delta = softplus(dt @ Wdt.T + bdt)."""
                    gb, cols = GROUPS[g], COLS[g]
                    st = mstate[g]
                    xact = st["xact"]
                    dbcp = psC.tile([128, 48], F32, tag="ps48", name=f"dbcp{g}")
                    for i in range(DIT):
                        nc.tensor.matmul(
                            dbcp[0:cols, :], xact[:, i, :cols], wxT[:, i, :],
                            start=(i == 0), stop=(i == DIT - 1),
                        )
                    dbc_sb = mam.tile([CMAX, 48], F32, tag="dbc", name=f"dbc{g}")
                    nc.vector.tensor_copy(out=dbc_sb[:cols, :], in_=dbcp[0:cols, :])
                    st["dbc"] = dbc_sb

                    dtp = psC.tile([128, 48], F32, tag="ps48", name=f"dtp{g}")
                    nc.tensor.transpose(
                        dtp[0:DTR, :cols], dbc_sb[:cols, 0:DTR], ident[0:cols, 0:cols]
                    )
                    dT_sb = mam.tile([DTR, CMAX], F32, tag="dT", name=f"dT{g}")
                    nc.vector.tensor_copy(out=dT_sb[:, :cols], in_=dtp[0:DTR, :cols])
                    delta = mam.tile([128, DIT, CMAX], F32, tag="delta", name=f"delta{g}")
                    for i in range(DIT):
                        dp = psC.tile([128, 48], F32, tag="ps48", name=f"dp{g}_{i}")
                        nc.tensor.matmul(
                            dp[:, :cols], wdtT[:, i * 128:(i + 1) * 128],
                            dT_sb[:, :cols],
                            start=True, stop=True,
                        )
                        nc.scalar.activation(
                            out=delta[:, i, :cols], in_=dp[:, :cols], func=AF.Exp,
                            bias=bdt2[:, i:i + 1],
                        )
                    nc.scalar.activation(
                        out=delta[:, :, :cols], in_=delta[:, :, :cols], func=AF.Ln,
                        bias=1.0,
                    )
                    st["delta"] = delta

                def mamba_B1(g):
                    """dA, dBu, selective scan, y2. Elementwise work runs on
                    GPSIMD so the DVE queue stays free for pool reduces."""
                    gb, cols = GROUPS[g], COLS[g]
                    st = mstate[g]
                    xact, delta, dbc_sb = st["xact"], st["delta"], st["dbc"]
                    cds = cols * DS
                    dA = mam.tile([128, DIT, CMAX * DS], BF16, tag="dA", name=f"dA{g}")
                    for i in range(DIT):
                        nc.gpsimd.tensor_tensor(
                            out=dA[:, i, :cds].rearrange(
                                "p (l jj n) -> p l jj n", jj=gb, n=DS
                            ),
                            in0=delta[:, i, :cols]
                            .rearrange("p (l jj) -> p l jj", jj=gb)
                            .unsqueeze(3)
                            .broadcast_to([128, L, gb, DS]),
                            in1=A3[:, i, :]
                            .unsqueeze(1)
                            .unsqueeze(1)
                            .broadcast_to([128, L, gb, DS]),
                            op=ALU.mult,
                        )
                    nc.scalar.activation(
                        out=dA[:, :, :cds], in_=dA[:, :, :cds], func=AF.Exp
                    )

                    du = mam.tile([128, DIT, CMAX], F32, tag="du", name=f"du{g}")
                    nc.gpsimd.tensor_tensor(
                        out=du[:, :, :cols], in0=delta[:, :, :cols],
                        in1=xact[:, :, :cols], op=ALU.mult,
                    )
                    bp_flat = mam.tile(
                        [1, CMAX * DS], F32, tag="flat", bufs=2, name=f"bpf{g}"
                    )
                    nc.scalar.dma_start(
                        out=bp_flat[:, :cds].rearrange("o (pb n) -> o pb n", n=DS),
                        in_=dbc_sb[:cols, DTR:DTR + DS],
                    )
                    dBu = mam.tile([128, DIT, CMAX * DS], BF16, tag="dBu", name=f"dBu{g}")
                    bpb = psB.tile([128, NCT, 64], F32, tag="bc", name=f"bpb{g}")
                    bpbf = bpb.rearrange("p a b -> p (a b)")
                    for k in range((cds + 511) // 512):
                        sl = slice(512 * k, min(cds, 512 * (k + 1)))
                        nc.tensor.matmul(
                            bpbf[:, sl], ones_row, bp_flat[:, sl], start=True, stop=True
                        )
                    bpsb = mam.tile([128, CMAX * DS], F32, tag="bpsb", name=f"bpsb{g}")
                    nc.vector.tensor_copy(out=bpsb[:, :cds], in_=bpbf[:, :cds])
                    for i in range(DIT):
                        nc.gpsimd.tensor_tensor(
                            out=dBu[:, i, :cds].rearrange(
                                "p (l jj n) -> p l jj n", jj=gb, n=DS
                            ),
                            in0=du[:, i, :cols]
                            .rearrange("p (l jj) -> p l jj", jj=gb)
                            .unsqueeze(3)
                            .broadcast_to([128, L, gb, DS]),
                            in1=bpsb[:, :cds].rearrange(
                                "p (l jj n) -> p l jj n", jj=gb, n=DS
                            ),
                            op=ALU.mult,
                        )
                    cp_flat = mam.tile(
                        [1, CMAX * DS], F32, tag="flat", bufs=2, name=f"cpf{g}"
                    )
                    nc.scalar.dma_start(
                        out=cp_flat[:, :cds].rearrange("o (pb n) -> o pb n", n=DS),
                        in_=dbc_sb[:cols, DTR + DS:DTR + 2 * DS],
                    )
                    cpb_ps = psB.tile([128, NCT, 64], F32, tag="bc", name=f"cpp{g}")
                    cpb_psf = cpb_ps.rearrange("p a b -> p (a b)")
                    for k in range((cds + 511) // 512):
                        sl = slice(512 * k, min(cds, 512 * (k + 1)))
                        nc.tensor.matmul(
                            cpb_psf[:, sl], ones_row, cp_flat[:, sl],
                            start=True, stop=True,
                        )
                    cpb = mam.tile([128, CMAX * DS], BF16, tag="cpb", name=f"cpb{g}")
                    nc.vector.tensor_copy(out=cpb[:, :cds], in_=cpb_psf[:, :cds])

                    # selective scan (in place on dBu): H_l += dA_l * H_{l-1}
                    for l in range(1, L):
                        blk = slice(l * gb * DS, (l + 1) * gb * DS)
                        prv = slice((l - 1) * gb * DS, l * gb * DS)
                        sc = mam.tile(
                            [128, DIT, 3 * DS], F32, tag="sc", bufs=2,
                            name=f"sc{g}_{l}",
                        )
                        nc.gpsimd.tensor_tensor(
                            out=sc[:, :, :gb * DS], in0=dA[:, :, blk],
                            in1=dBu[:, :, prv], op=ALU.mult,
                        )
                        nc.gpsimd.tensor_tensor(
                            out=dBu[:, :, blk], in0=dBu[:, :, blk],
                            in1=sc[:, :, :gb * DS], op=ALU.add,
                        )
                    # y = sum_n H * Cp  (dA reused as scratch), tree-reduced
                    nc.gpsimd.tensor_tensor(
                        out=dA[:, :, :cds], in0=dBu[:, :, :cds],
                        in1=cpb[:, :cds].unsqueeze(1).broadcast_to([128, DIT, cds]),
                        op=ALU.mult,
                    )
                    dAv = dA[:, :, :cds].rearrange("p i (c n) -> p i c n", n=DS)
                    yt = mam.tile([128, DIT, CMAX, 8], F32, tag="dBu", name=f"yt{g}")
                    nc.gpsimd.tensor_tensor(
                        out=yt[:, :, :cols, :], in0=dAv[:, :, :, 0:8],
                        in1=dAv[:, :, :, 8:16], op=ALU.add,
                    )
                    nc.gpsimd.tensor_tensor(
                        out=yt[:, :, :cols, 0:4], in0=yt[:, :, :cols, 0:4],
                        in1=yt[:, :, :cols, 4:8], op=ALU.add,
                    )
                    nc.gpsimd.tensor_tensor(
                        out=yt[:, :, :cols, 0:2], in0=yt[:, :, :cols, 0:2],
                        in1=yt[:, :, :cols, 2:4], op=ALU.add,
                    )
                    y2 = mam.tile([128, DIT, CMAX], BF16, tag="y2", name=f"y2{g}")
                    nc.gpsimd.tensor_tensor(
                        out=y2[:, :, :cols], in0=yt[:, :, :cols, 0],
                        in1=yt[:, :, :cols, 1], op=ALU.add,
                    )
                    # y2 += x * D_ssm (e_t reused as scratch)
                    xd = mstate[g]["xact"]
                    et = mam.tile([128, DIT, CMAX], F32, tag="e_t", name=f"etd{g}")
                    nc.gpsimd.tensor_tensor(
                        out=et[:, :, :cols], in0=xd[:, :, :cols],
                        in1=dssm2.unsqueeze(2).broadcast_to([128, DIT, cols]),
                        op=ALU.mult,
                    )
                    nc.gpsimd.tensor_tensor(
                        out=y2[:, :, :cols], in0=y2[:, :, :cols],
                        in1=et[:, :, :cols], op=ALU.add,
                    )
                    st["y2"] = y2

                def mamba_B2(g):
                    """vi2 = y2 @ W_out.T + Vi/48; LN2; channel attention."""
                    gb, cols = GROUPS[g], COLS[g]
                    st = mstate[g]
                    y2 = st["y2"]
                    vi2 = vip.tile([128, NCT, CMAX], F32, tag="v2", bufs=2,
                                   name=f"vi2{g}")
                    vi2p = psB.tile([128, NCT, 64], F32, tag="bc", name=f"vi2p{g}")
                    for mc in range(NCT):
                        for i in range(DIT):
                            nc.tensor.matmul(
                                vi2p[:, mc, :cols],
                                woutT[:, i, mc * 128:(mc + 1) * 128], y2[:, i, :cols],
                                start=(i == 0), stop=(i == DIT - 1),
                            )
                    nc.vector.scalar_tensor_tensor(
                        out=vi2[:, :, :cols], in0=Vi_g[g][:, :, :cols],
                        scalar=1.0 / POOL_W,
                        in1=vi2p[:, :, :cols], op0=ALU.mult, op1=ALU.add,
                    )
                    nrm = vip.tile([128, NCT, CMAX], F32, tag="v2", bufs=2,
                                   name=f"nrm{g}")
                    layer_norm(vi2, ln2g, ln2b, nrm, cols, f"b{g}")

                    for s in range(2):
                        mv = mam.tile([128, NCT, 3], BF16, tag="mv", name=f"mv{g}{s}")
                        mx = mam.tile([128, NCT, 3], BF16, tag="mx", name=f"mx{g}{s}")
                        view = nrm[:, :, :cols].rearrange(
                            "p c (pp two jj) -> p c two jj pp", two=2, jj=gb
                        )[:, :, s, :, :]
                        with nc.allow_low_precision(reason="bf16 attention pool"):
                            nc.vector.reduce_sum(out=mv[:, :, :gb], in_=view,
                                                 axis=AX.X)
                            nc.vector.reduce_max(out=mx[:, :, :gb], in_=view,
                                                 axis=AX.X)
                        h1m = mam.tile([128, 3], BF16, tag="h1m", name=f"h1m{g}{s}")
                        h1x = mam.tile([128, 3], BF16, tag="h1x", name=f"h1x{g}{s}")
                        for src, dst, sc_ap in ((mv, h1m, absm), (mx, h1x, absx)):
                            hp = psC.tile([128, 48], F32, tag="ps48", name=f"hp{g}{s}")
                            for ci in range(NCT):
                                nc.tensor.matmul(
                                    hp[:, 0:gb], aw1T[:, ci, :], src[:, ci, :gb],
                                    start=(ci == 0), stop=(ci == NCT - 1),
                                )
                            nc.scalar.activation(
                                out=dst[:, :gb], in_=hp[:, 0:gb], func=AF.Relu,
                                scale=sc_ap, bias=abnb,
                            )
                        att = att_g[g][s]
                        apb = psB.tile([128, NCT, 64], F32, tag="bc", name=f"apb{g}{s}")
                        for mc in range(NCT):
                            nc.tensor.matmul(
                                apb[:, mc, 0:gb], aw2T[:, mc * 128:(mc + 1) * 128],
                                h1m[:, :gb],
                                start=True, stop=False,
                            )
                            nc.tensor.matmul(
                                apb[:, mc, 0:gb], aw2T[:, mc * 128:(mc + 1) * 128],
                                h1x[:, :gb],
                                start=False, stop=True,
                            )
                        # sigmoid = 1/(1+exp(-x)); the tail runs on GPSIMD so
                        # the single natural_log_exp act table serves all acts
                        nc.scalar.activation(
                            out=att[:, :, :], in_=apb[:, :, 0:gb], func=AF.Exp,
                            scale=-1.0,
                        )
                        ones_att = ones_col.unsqueeze(2).broadcast_to(
                            [128, NCT, GROUPS[g]]
                        )
                        nc.gpsimd.tensor_tensor(
                            out=att[:, :, :], in0=att[:, :, :], in1=ones_att,
                            op=ALU.add,
                        )
                        nc.vector.reciprocal(out=att[:, :, :], in_=att[:, :, :])

                def mamba_full(g):
                    mamba_A1(g)
                    mamba_A2(g)
                    mamba_B1(g)
                    mamba_B2(g)

                # ========================================================
                # FFN: load fm, cast to bf16 ax staging, scale by gate,
                # paired matmuls, relu, store.
                # ========================================================
                ax_t = {}

                def load_pair(pi, s):
                    """Issue the fm loads for both batches of the pair."""
                    fts = {}
                    for h, b in enumerate(PAIRS[pi]):
                        for cq in range(4):
                            ft = fmp.tile(
                                [128, 4, HW], F32, tag="fm",
                                name=f"ffm{s}_{b}_{cq}",
                            )
                            nc.sync.dma_start(
                                out=ft,
                                in_=fm_d[s][b, cq * 512:(cq + 1) * 512, :]
                                .rearrange("(p four) w -> p four w", four=4),
                            )
                            fts[(h, cq)] = ft
                    return fts

                def cast_pair(pi, s, fts, eng="mix"):
                    """Cast fm tiles into ax staging [128, 4, 576]."""
                    ax4 = [
                        axp.tile([128, 4, 2 * HW], BF16, tag="ax", bufs=8,
                                 name=f"ax{s}_{pi}_{cq}")
                        for cq in range(4)
                    ]
                    ax_t[(pi, s)] = ax4
                    for h in range(2):
                        for cq in range(4):
                            dst = ax4[cq][:, :, h * HW:(h + 1) * HW]
                            ft = fts[(h, cq)]
                            if eng == "gps":
                                nc.gpsimd.tensor_copy(out=dst, in_=ft)
                            elif cq % 2 == 0:
                                nc.vector.tensor_copy(out=dst, in_=ft)
                            else:
                                nc.scalar.activation(
                                    out=dst, in_=ft, func=AF.Copy
                                )

                def load_cast_pair(pi, s, skip_io=False, eng="mix"):
                    if skip_io:
                        ax4 = [
                            axp.tile([128, 4, 2 * HW], BF16, tag="ax", bufs=8,
                                     name=f"ax{s}_{pi}_{cq}")
                            for cq in range(4)
                        ]
                        ax_t[(pi, s)] = ax4
                        for cq in range(4):
                            nc.gpsimd.memset(ax4[cq], 0.01)
                        return
                    cast_pair(pi, s, load_pair(pi, s), eng=eng)

                def scale_pair(pi, s):
                    """In-place gate on GPSIMD: ax *= att (per channel/batch)."""
                    ax4 = ax_t[(pi, s)]
                    for h, b in enumerate(PAIRS[pi]):
                        g = B2G[b]
                        j = b - GSTART[g]
                        att = att_g[g][s]
                        for cq in range(4):
                            nc.gpsimd.tensor_tensor(
                                out=ax4[cq][:, :, h * HW:(h + 1) * HW],
                                in0=ax4[cq][:, :, h * HW:(h + 1) * HW],
                                in1=att[:, cq * 4:(cq + 1) * 4, j:j + 1]
                                .broadcast_to([128, 4, HW]),
                                op=ALU.mult,
                            )

                def mm_pair(pi, s, hooks=None, skip_io=False):
                    ax4 = ax_t.pop((pi, s))
                    wt = wts[s]
                    b0, b1 = PAIRS[pi]
                    for mq in range(4):
                        ot0 = otp.tile([128, 4, HW], BF16, tag="ot",
                                       name=f"ot{s}_{b0}_{mq}")
                        ot1 = otp.tile([128, 4, HW], BF16, tag="ot",
                                       name=f"ot{s}_{b1}_{mq}")
                        for mi in range(4):
                            mc = mq * 4 + mi
                            pp0 = psA.tile([128, HW], F32, tag="pp",
                                           name=f"pp{s}_{pi}_{mc}a")
                            pp1 = psA.tile([128, HW], F32, tag="pp",
                                           name=f"pp{s}_{pi}_{mc}b")
                            for kc in range(NCT):
                                nc.tensor.matmul(
                                    pp0, wt[mq][kc][:, mi * 128:(mi + 1) * 128],
                                    ax4[kc // 4][:, kc % 4, 0:HW],
                                    start=(kc == 0), stop=(kc == NCT - 1),
                                )
                            for kc in range(NCT):
                                nc.tensor.matmul(
                                    pp1, wt[mq][kc][:, mi * 128:(mi + 1) * 128],
                                    ax4[kc // 4][:, kc % 4, HW:2 * HW],
                                    start=(kc == 0), stop=(kc == NCT - 1),
                                )
                            nc.scalar.activation(
                                out=ot0[:, mi, :], in_=pp0, func=AF.Relu,
                                scale=scl_s[s][:, mc:mc + 1],
                                bias=scl_b[s][:, mc:mc + 1],
                            )
                            nc.scalar.activation(
                                out=ot1[:, mi, :], in_=pp1, func=AF.Relu,
                                scale=scl_s[s][:, mc:mc + 1],
                                bias=scl_b[s][:, mc:mc + 1],
                            )
                        if not skip_io:
                            nc.scalar.dma_start(
                                out=out_d[s][b0, mq * 512:(mq + 1) * 512, :]
                                .rearrange("(p four) w -> p four w", four=4),
                                in_=ot0,
                            )
                            nc.scalar.dma_start(
                                out=out_d[s][b1, mq * 512:(mq + 1) * 512, :]
                                .rearrange("(p four) w -> p four w", four=4),
                                in_=ot1,
                            )
                        if hooks and mq in hooks:
                            for fn in hooks[mq]:
                                fn()

                # ========================================================
                # emission
                # ========================================================
                if parts == "front":
                    pool_loads(0)
                    mamba_A1(0, per_ci=True)
                    pool_loads(1)
                    pool_loads(2)
                    mamba_A2(0)
                    mamba_B1(0)
                    mamba_B2(0)
                    for g in range(1, NG):
                        mamba_full(g)
                elif parts in ("ffn", "mm"):
                    for g in range(NG):
                        for s in range(2):
                            nc.vector.memset(att_g[g][s], 1.0)
                    skip = parts == "mm"
                    load_w(0)
                    load_cast_pair(0, 0, skip_io=skip)
                    load_cast_pair(1, 0, skip_io=skip)
                    for s in range(2):
                        for pi in range(4):
                            scale_pair(pi, s)
                            nxt = (pi + 2, s) if pi < 2 else (pi - 2, s + 1)
                            hooks = None
                            if nxt[1] < 2:
                                hooks = {3: [
                                    (lambda p=nxt[0], ss=nxt[1]:
                                     load_cast_pair(p, ss, skip_io=skip))
                                ]}
                            mm_pair(pi, s, hooks=hooks, skip_io=skip)
                            if s == 0 and pi == 3:
                                load_w(1)
                else:
                    pool_loads(0)
                    mamba_A1(0, per_ci=True)
                    # pair-0 loads + gps casts sit before mamba B1's gps ops
                    # so staging slots free early without touching DVE/Act.
                    f0 = load_pair(0, 0)
                    cast_pair(0, 0, f0, eng="gps")
                    load_w(0, mqs=(0,))
                    mamba_A2(0)
                    mamba_B1(0)
                    mamba_B2(0)
                    pool_loads(1)
                    load_w(0, mqs=(1, 2, 3))
                    f1 = load_pair(1, 0)
                    g2_reds = pool_loads(2, defer_reds=True)
                    scale_pair(0, 0)
                    cast_pair(1, 0, f1, eng="gps")

                    def emit_reds(rs):
                        for r in rs:
                            r()

                    mm_pair(0, 0, hooks={
                        0: [lambda: mamba_A1(1)],
                        1: [lambda: mamba_A2(1), lambda: mamba_B1(1)],
                        2: [lambda: mamba_B2(1),
                            lambda: emit_reds(g2_reds[:12])],
                        3: [lambda: emit_reds(g2_reds[12:]),
                            lambda: load_cast_pair(2, 0)],
                    })
                    scale_pair(1, 0)
                    mm_pair(1, 0, hooks={
                        0: [lambda: mamba_A1(2)],
                        1: [lambda: mamba_A2(2), lambda: mamba_B1(2)],
                        2: [lambda: mamba_B2(2)],
                        3: [lambda: load_cast_pair(3, 0)],
                    })
                    scale_pair(2, 0)
                    mm_pair(2, 0, hooks={3: [lambda: load_cast_pair(0, 1)]})
                    scale_pair(3, 0)
                    mm_pair(3, 0, hooks={3: [lambda: load_cast_pair(1, 1)]})
                    load_w(1)
                    scale_pair(0, 1)
                    mm_pair(0, 1, hooks={3: [lambda: load_cast_pair(2, 1)]})
                    scale_pair(1, 1)
                    mm_pair(1, 1, hooks={3: [lambda: load_cast_pair(3, 1)]})
                    scale_pair(2, 1)
                    mm_pair(2, 1)
                    scale_pair(3, 1)
                    mm_pair(3, 1)

    nc.compile()
    return nc


# channel permutation: K-tile kc, partition p holds channel (kc//4)*512 + 4*p + (kc%4)
# so each DMA descriptor covers 4 consecutive channels (4.6KB contiguous).
_PERM = np.array(
    [[(kc // 4) * 512 + 4 * p + (kc % 4) for p in range(128)] for kc in range(NCT)]
).reshape(-1)  # [2048] in (kc, p) order


def _host_prep(inputs):
    """Host-side weight layout prep. Returns dict of per-core-replicated arrays."""
    f32 = np.float32
    bf16 = ml_dtypes.bfloat16
    g = lambda k: np.asarray(inputs[k], dtype=f32)
    s_bn = f32(1.0 / np.sqrt(1.0 + EPS))

    def ctile(v):  # [C] -> [128, 16], channel-permuted
        return np.ascontiguousarray(v[_PERM].reshape(NCT, 128).T)

    def dtile(v):  # [DI] -> [128, 2]
        return np.ascontiguousarray(v.reshape(DIT, 128).T)

    A = -np.exp(g("A_log"))  # [256, 16]
    sm_parts = {
        "wx": g("Wx").T.reshape(DIT, 128, 48).transpose(1, 0, 2).reshape(128, -1),
        "cw": g("conv_w")[:, 0, :].reshape(DIT, 128, 3).transpose(1, 0, 2).reshape(128, -1),
        "ncb": dtile(-g("conv_b")),
        "cb": dtile(g("conv_b")),
        "bdt": dtile(g("bdt")),
        "dssm": dtile(g("D_ssm")),
        "A3": A.reshape(DIT, 128, DS).transpose(1, 0, 2).reshape(128, -1),
        "ln1g": ctile(g("ln1_g")), "ln1b": ctile(g("ln1_b")),
        "ln2g": ctile(g("ln2_g")), "ln2b": ctile(g("ln2_b")),
        "absm": (g("att_bn_g") * s_bn / 6.0)[:, None],
        "absx": (g("att_bn_g") * s_bn)[:, None],
        "abnb": g("att_bn_b")[:, None],
        "fvs": ctile(g("ffn_vis_bn_g") * s_bn),
        "fvb": ctile(g("ffn_vis_b") * (g("ffn_vis_bn_g") * s_bn) + g("ffn_vis_bn_b")),
        "fis": ctile(g("ffn_inf_bn_g") * s_bn),
        "fib": ctile(g("ffn_inf_b") * (g("ffn_inf_bn_g") * s_bn) + g("ffn_inf_bn_b")),
    }
    smalls = np.zeros((128, SM_COLS), f32)
    for name, _w in SMALLS:
        a, b = SM_OFF[name]
        smalls[:, a:b] = sm_parts[name]

    prep = {
        "smalls": smalls,
        "w_inT": np.ascontiguousarray(
            g("W_in").T[_PERM].reshape(NCT, 128, DI).transpose(1, 0, 2)
        ).astype(bf16),
        "wdtT": np.ascontiguousarray(g("Wdt").T),
        "w_outT": np.ascontiguousarray(
            g("W_out").T[:, _PERM].reshape(DIT, 128, C).transpose(1, 0, 2)
        ).astype(bf16),
        "aw1T": np.ascontiguousarray(
            g("att_w1").T[_PERM].reshape(NCT, 128, 128).transpose(1, 0, 2)
        ).astype(bf16),
        "aw2T": np.ascontiguousarray(g("att_w2").T[:, _PERM]).astype(bf16),
        "wvT": np.ascontiguousarray(
            g("ffn_vis_w").T[_PERM][:, _PERM].reshape(NCT, 128, C).transpose(1, 0, 2)
        ).astype(bf16),
        "wiT": np.ascontiguousarray(
            g("ffn_inf_w").T[_PERM][:, _PERM].reshape(NCT, 128, C).transpose(1, 0, 2)
        ).astype(bf16),
    }
    return prep


def _get_runner():
    """Build the bass program once and wrap it in a reusable jitted callable."""
    if "runner" in _CACHE:
        return _CACHE["runner"]

    import jax
    import numpy as _np
    from jax.sharding import Mesh, PartitionSpec
    from jax.experimental.shard_map import shard_map
    import concourse.bacc as bacc
    import concourse.tile as tile
    from concourse import mybir, masks
    from concourse import bass2jax

    nc = _build(bacc, tile, mybir, masks)
    bass2jax.install_neuronx_cc_hook()

    pname = nc.partition_id_tensor.name if nc.partition_id_tensor else None
    in_names, out_names, out_avals, zero_shapes = [], [], [], []
    for alloc in nc.m.functions[0].allocations:
        if not isinstance(alloc, mybir.MemoryLocationSet):
            continue
        name = alloc.memorylocations[0].name
        if alloc.kind == "ExternalInput":
            if name != pname:
                in_names.append(name)
        elif alloc.kind == "ExternalOutput":
            out_names.append(name)
            shape = tuple(alloc.tensor_shape)
            dtype = mybir.dt.np(alloc.dtype)
            out_avals.append(jax.core.ShapedArray(shape, dtype))
            zero_shapes.append((shape, dtype))
    n_params = len(in_names)
    all_names = list(in_names) + list(out_names)
    if pname is not None:
        all_names.append(pname)

    def _body(*args):
        operands = list(args)
        if pname is not None:
            operands.append(bass2jax.partition_id_tensor())
        outs = bass2jax._bass_exec_p.bind(
            *operands,
            out_avals=tuple(out_avals),
            in_names=tuple(all_names),
            out_names=tuple(out_names),
            lowering_input_output_aliases=(),
            sim_require_finite=False,
            sim_require_nnan=False,
            nc=nc,
        )
        return tuple(outs)

    devices = jax.devices()[:N_CORES]
    mesh = Mesh(_np.asarray(devices), ("core",))
    specs = (PartitionSpec("core"),) * (n_params + len(out_names))
    fn = jax.jit(
        shard_map(
            _body,
            mesh=mesh,
            in_specs=specs,
            out_specs=(PartitionSpec("core"),) * len(out_names),
            check_rep=False,
        ),
        keep_unused=True,
    )
    runner = {
        "fn": fn,
        "in_names": in_names,
        "out_names": out_names,
        "zero_shapes": zero_shapes,
        "nc": nc,
    }
    _CACHE["runner"] = runner
    return runner


def kernel(**inputs):
    runner = _get_runner()
    prep = _host_prep(inputs)
    vis = np.asarray(inputs["vis_feat_map"], dtype=np.float32).reshape(B_FULL, C, HW)
    inf = np.asarray(inputs["inf_feat_map"], dtype=np.float32).reshape(B_FULL, C, HW)

    # global inputs: concat of per-core shards along axis 0
    per_in = {"vis": vis, "inf": inf}  # already [64, ...] = 8 cores x [8, ...]
    gin = []
    for name in runner["in_names"]:
        if name in per_in:
            gin.append(per_in[name])
        else:
            arr = prep[name]
            gin.append(np.broadcast_to(arr, (N_CORES,) + arr.shape).reshape(
                (N_CORES * arr.shape[0],) + arr.shape[1:]
            ))
    zeros = [
        np.zeros((N_CORES * s[0],) + tuple(s[1:]), dt)
        for (s, dt) in runner["zero_shapes"]
    ]
    outs = runner["fn"](*gin, *zeros)
    res = {
        name: np.asarray(outs[i]) for i, name in enumerate(runner["out_names"])
    }
    out_vis = res["out_vis"].astype(np.float32).reshape(B_FULL, C, H, W)
    out_inf = res["out_inf"].astype(np.float32).reshape(B_FULL, C, H, W)
    return (out_vis, out_inf)
